# revision 18
# baseline (speedup 1.0000x reference)
"""Trainium2 Bass kernel for nn_CBDLoss (path-affinity cross-entropy loss).

Key insight: the int64 "gather" tables are just shifted 247x238 crop windows of
the 256x256 image. Each of the 152 path channels' affinity is
    aff[b, p, m] = 1 - max over path atoms (dy,dx) of image[b, r+dy, 9+dx+c]
with m = r*238 + c. So the whole gather+maxpool is a set of window-shifted
elementwise maxes, shareable across paths via a prefix trie (727 max ops
instead of 2134 raw path-steps).

Sharding (uniform SPMD, one program for all 8 cores): split the output grid by
(batch b, row-tile t, col-half h) -> 2*2*2 = 8 cores. Every core runs the same
global trie on its [128 rows x 120 cols] slice; padded rows/cols are masked by
zero labels (log terms stay finite, so 0*log = 0 contributes nothing).

Loss reduction is fused on-chip:
  logpos = Ln(-mx + (1+eps)),  logneg = Ln(mx + eps)        (ScalarE, LUT)
  numer += sum(label * log...)  via tensor_tensor_reduce     (VectorE, fused)
  denom += sum(label)           via activation Copy+accum    (ScalarE, fused)
Each core returns a [128, 32] block of per-partition partial sums; the host
does the tiny final reduction and the scalar loss formula.
"""

import os
import sys

import numpy as np

for _p in ("/opt/trn_rl_repo", "/root/.axon_site/_ro/trn_rl_repo"):
    if os.path.isdir(_p) and _p not in sys.path:
        sys.path.insert(0, _p)

RADIUS = 10
CROP = 256
EPS = 1e-5
B = 2
RF = RADIUS - 1          # 9
CH = CROP - RF           # 247 output rows
CW = CROP - 2 * RF       # 238 output cols
M = CH * CW              # 58786

NCORES = 8
NPATH = 152
NG = 8                   # path groups
GSZ = NPATH // NG        # 19 paths per group
PF = 120                 # per-path free width on a core (119 valid + 1 pad col)
GF = GSZ * PF            # 2280
COLS_H = 119             # valid cols per col-half
IMG_R = 137              # per-core image slice rows (128 + 9 halo)
IMG_C = 138              # per-core image slice cols (120 + 18 halo)
OUTW = 4 * NG            # out columns: [ttr_pos | ttr_neg | den_pos | den_neg]


def _gen_paths():
    """Replicates reference._get_all_dir_paths ordering -> 152 atom lists."""
    by_len = [[] for _ in range(RADIUS * 4)]
    search_dirs = [(0, x) for x in range(1, RADIUS)]
    for y in range(1, RADIUS):
        for x in range(-RADIUS + 1, RADIUS):
            if x * x + y * y < RADIUS ** 2:
                search_dirs.append((y, x))
    for d in search_dirs:
        length_sq = d[0] ** 2 + d[1] ** 2
        coords = []
        min_y, max_y = sorted((0, d[0]))
        min_x, max_x = sorted((0, d[1]))
        for y in range(min_y, max_y + 1):
            for x in range(min_x, max_x + 1):
                if (d[0] * x - d[1] * y) ** 2 / length_sq < 1:
                    coords.append((y, x))
        coords.sort(key=lambda c: -abs(c[0]) - abs(c[1]))
        by_len[len(coords)].append(coords)
    paths = []
    for g in by_len:
        paths.extend(g)
    assert len(paths) == NPATH
    return paths


def _trie():
    """Build the shared-prefix trie over canonically-ordered atom lists.

    Returns (order, term_of_node, perm):
      order: DFS list of op-nodes (tuples of atoms, depth >= 2)
      term_of_node: node -> terminal slot index (DFS terminal order) or None
      perm: perm[slot] = original path index whose labels go in that slot
    """
    paths = _gen_paths()
    cpaths = [tuple(sorted(p, key=lambda c: (abs(c[0]) + abs(c[1]), c[0], c[1])))
              for p in paths]
    nodes = set()
    for cp in cpaths:
        for i in range(2, len(cp) + 1):
            nodes.add(cp[:i])
    children = {}
    for n in nodes:
        children.setdefault(n[:-1], []).append(n)
    for k in children:
        children[k].sort()
    term_set = set(cpaths)
    assert len(term_set) == NPATH

    order = []
    term_order = []
    stack = list(reversed(children.get(((0, 0),), [])))
    # iterative DFS preserving recursive order
    def dfs(node):
        order.append(node)
        if node in term_set:
            term_order.append(node)
        for c in children.get(node, []):
            dfs(c)
    sys.setrecursionlimit(10000)
    for c in children.get(((0, 0),), []):
        dfs(c)
    assert len(order) == len(nodes) and len(term_order) == NPATH

    term_idx = {t: i for i, t in enumerate(term_order)}
    term_of_node = {n: term_idx.get(n) for n in order}
    perm = sorted(range(NPATH), key=lambda j: term_idx[cpaths[j]])
    return order, term_of_node, perm


_PERM = None
_PROG = None


def _build_program():
    import concourse.bass as bass
    import concourse.mybir as mybir
    import concourse.tile as tile

    dt = mybir.dt
    f32 = dt.float32
    order, term_of_node, _ = _trie()

    nc = bass.Bass()
    img = nc.declare_dram_parameter("img", [IMG_R, IMG_C], f32, isOutput=False)
    pos_lab = nc.declare_dram_parameter("pos_lab", [NG, 128, GF], f32, isOutput=False)
    neg_lab = nc.declare_dram_parameter("neg_lab", [NG, 128, GF], f32, isOutput=False)
    out = nc.declare_dram_parameter("out", [128, OUTW], f32, isOutput=True)

    with tile.TileContext(nc) as tc:
        from contextlib import ExitStack
        with ExitStack() as ctx:
            const_pool = ctx.enter_context(tc.tile_pool(name="const", bufs=1))
            trie_pool = ctx.enter_context(tc.tile_pool(name="trie", bufs=24))
            mxg_pool = ctx.enter_context(tc.tile_pool(name="mxg", bufs=2))
            lab_pool = ctx.enter_context(tc.tile_pool(name="lab", bufs=2))
            log_pool = ctx.enter_context(tc.tile_pool(name="log", bufs=2))
            dsc_pool = ctx.enter_context(tc.tile_pool(name="dsc", bufs=2))
            accs_pool = ctx.enter_context(tc.tile_pool(name="accs", bufs=8))

            # All 10 row-shifted image copies in ONE tile via ONE dma:
            # imgall[p, dy*IMG_C + c] = img[p+dy, c]  (affine src pattern).
            imgall = const_pool.tile([128, RADIUS * IMG_C], f32, tag="imgall")
            # src AP dims (p-major to match the SBUF dst):
            #   [p (stride IMG_C), dy (stride IMG_C), c (stride 1)]
            # -> overlapping windows img[p+dy, c], built directly as an AP.
            from concourse.bass_types import AP as BassAP
            img_src = BassAP(img[:, :].tensor, 0,
                             [[IMG_C, 128], [IMG_C, RADIUS], [1, IMG_C]])
            nc.sync.dma_start(
                imgall[:, :].rearrange("p (o c) -> p o c", o=RADIUS),
                img_src)
            bias_pos = const_pool.tile([128, 1], f32, tag="bias_pos")
            nc.vector.memset(bias_pos[:, :], 1.0 + EPS)
            bias_neg = const_pool.tile([128, 1], f32, tag="bias_neg")
            nc.vector.memset(bias_neg[:, :], EPS)

            # write-only dump tiles for the mandatory full-size outputs of
            # TTR / Copy+accum; allocated once so no pool-slot recycling
            # (slot reuse makes Tile emit extra per-instruction sem waits,
            # and walrus caps sync waits per instruction).
            dump_vp = const_pool.tile([128, GF], f32, tag="dump_vp")
            dump_vn = const_pool.tile([128, GF], f32, tag="dump_vn")

            acc_vp = const_pool.tile([128, NG], f32, tag="acc_vp")
            acc_vn = const_pool.tile([128, NG], f32, tag="acc_vn")
            acc_sp_tiles = []
            acc_sn_tiles = []

            def atom_ap(a):
                dy, dx = a
                s = dy * IMG_C + RF + dx
                return imgall[:, s:s + PF]

            Ln = mybir.ActivationFunctionType.Ln
            Copy = mybir.ActivationFunctionType.Copy
            mult = mybir.AluOpType.mult
            add = mybir.AluOpType.add

            node_val = {}
            mxg_tile = None

            def consume_group(g, mxg):
                labp = lab_pool.tile([128, GF], f32, tag="labp")
                nc.sync.dma_start(labp[:, :], pos_lab[g, :, :])
                labn = lab_pool.tile([128, GF], f32, tag="labn")
                nc.sync.dma_start(labn[:, :], neg_lab[g, :, :])

                logp = log_pool.tile([128, GF], f32, tag="logp")
                nc.scalar.activation(logp[:, :], mxg[:, :], Ln,
                                     bias=bias_pos[:, 0:1], scale=-1.0)
                logn = log_pool.tile([128, GF], f32, tag="logn")
                nc.scalar.activation(logn[:, :], mxg[:, :], Ln,
                                     bias=bias_neg[:, 0:1], scale=1.0)

                nc.vector.scalar_tensor_tensor(
                    out=dump_vp[:, :], in0=labp[:, :], scalar=1.0,
                    in1=logp[:, :], op0=mult, op1=mult,
                    accum_out=acc_vp[:, g:g + 1])
                nc.vector.scalar_tensor_tensor(
                    out=dump_vn[:, :], in0=labn[:, :], scalar=1.0,
                    in1=logn[:, :], op0=mult, op1=mult,
                    accum_out=acc_vn[:, g:g + 1])

                dsp = dsc_pool.tile([128, GF], f32, tag="dsp")
                asp = accs_pool.tile([128, 1], f32, tag="asp")
                nc.scalar.activation(dsp[:, :], labp[:, :], Copy,
                                     accum_out=asp[:, 0:1])
                acc_sp_tiles.append(asp)
                dsn = dsc_pool.tile([128, GF], f32, tag="dsn")
                asn = accs_pool.tile([128, 1], f32, tag="asn")
                nc.scalar.activation(dsn[:, :], labn[:, :], Copy,
                                     accum_out=asn[:, 0:1])
                acc_sn_tiles.append(asn)

            for node in order:
                parent = node[:-1]
                pin = atom_ap(parent[0]) if len(parent) == 1 else node_val[parent]
                ain = atom_ap(node[-1])
                slot = term_of_node[node]
                if slot is None:
                    dest_tile = trie_pool.tile([128, PF], f32, tag="trie")
                    dest = dest_tile[:, :]
                else:
                    g, s = divmod(slot, GSZ)
                    if s == 0:
                        mxg_tile = mxg_pool.tile([128, GF], f32, tag="mxg")
                    dest = mxg_tile[:, s * PF:(s + 1) * PF]
                nc.vector.tensor_max(dest, pin, ain)
                node_val[node] = dest
                if slot is not None and slot % GSZ == GSZ - 1:
                    consume_group(slot // GSZ, mxg_tile)

            nc.sync.dma_start(out[:, 0:NG], acc_vp[:, :])
            nc.sync.dma_start(out[:, NG:2 * NG], acc_vn[:, :])
            for g in range(NG):
                nc.sync.dma_start(out[:, 2 * NG + g:2 * NG + g + 1],
                                  acc_sp_tiles[g][:, 0:1])
                nc.sync.dma_start(out[:, 3 * NG + g:3 * NG + g + 1],
                                  acc_sn_tiles[g][:, 0:1])

    _hoist_excess_waits(nc, mybir)
    _replace_sem_range_clear(nc, mybir)
    return nc


def _replace_sem_range_clear(nc, mybir):
    """Tile's epilogue clears its semaphore range with a raw-ISA EVENT_SEM
    RANGE_CLEAR on Pool, which this walrus build rejects ("ISA wrong length").
    Replace it with per-semaphore EventSemaphore writes of 0."""
    for fn in nc.m.functions:
        for b in fn.blocks:
            out_list = []
            for inst in b.instructions:
                if type(inst).__name__ == "InstISA":
                    d = inst.ant_dict
                    if isinstance(d, dict) and "range_first" in d:
                        si = inst.sync_info
                        waits = list(si.on_wait) if si and si.on_wait else []
                        base_updates = list(si.on_update) if si and si.on_update else []
                        for k, sem in enumerate(
                                range(d["range_first"], d["range_last"] + 1)):
                            ev = mybir.InstEventSemaphore(
                                name=f"{inst.name}-semclr-{sem}")
                            ev.engine = inst.engine
                            upd = [mybir.SyncUpdate(
                                sync_type="semaphore", id=sem,
                                ant_name=f"semclr_{sem}",
                                update_mode="sem-wr-imm", update_value=0)]
                            if k == len(range(d["range_first"], d["range_last"] + 1)) - 1:
                                upd.extend(base_updates)
                            ev.sync_info = mybir.SyncInfo(
                                on_wait=waits if k == 0 else [],
                                on_update=upd)
                            out_list.append(ev)
                        continue
                out_list.append(inst)
            b.instructions[:] = out_list


def _hoist_excess_waits(nc, mybir):
    """Walrus caps semaphore waits per hardware instruction (1 for ACT, 2 for
    TT-family).  Move excess waits onto standalone NoOps on the same engine,
    inserted just before the over-limit instruction."""
    LIMITS = {"InstTensorTensor": 2, "InstTensorTensorReduce": 2,
              "InstTensorReduce": 2, "InstMatmult": 2,
              "InstCustomDveAnt": 2, "InstTensorScalarPtr": 1}
    uid = 0
    for fn in nc.m.functions:
        for b in fn.blocks:
            out_list = []
            for inst in b.instructions:
                si = getattr(inst, "sync_info", None)
                if si is not None and si.on_wait:
                    lim = LIMITS.get(type(inst).__name__, 1)
                    waits = list(si.on_wait)
                    if len(waits) > lim:
                        keep = waits[-lim:] if lim else []
                        for w in waits[:len(waits) - lim]:
                            nop = mybir.InstDrain(name=f"I-waithoist-{uid}")
                            uid += 1
                            nop.engine = inst.engine
                            nop.sync_info = mybir.SyncInfo(
                                on_wait=[w], on_update=[])
                            out_list.append(nop)
                        inst.sync_info = mybir.SyncInfo(
                            on_wait=keep, on_update=list(si.on_update or []))
                out_list.append(inst)
            b.instructions[:] = out_list


def _get_program():
    global _PROG, _PERM
    if _PROG is None:
        _, _, _PERM = _trie()
        _PROG = _build_program()
    return _PROG


def _host_prep(pred_output, pos_label, neg_label):
    """Build per-core input maps."""
    global _PERM
    image = np.ascontiguousarray(
        np.asarray(pred_output, dtype=np.float32).reshape(B, CROP, CROP))
    labs = [np.asarray(pos_label, dtype=np.float32).reshape(B, NPATH, CH, CW),
            np.asarray(neg_label, dtype=np.float32).reshape(B, NPATH, CH, CW)]
    perm = _PERM

    in_maps = []
    for core in range(NCORES):
        b, t, h = core >> 2, (core >> 1) & 1, core & 1
        r0, c0 = t * 128, h * COLS_H

        img_core = np.zeros((IMG_R, IMG_C), np.float32)
        nr = min(IMG_R, CROP - r0)
        ncol = min(IMG_C, CROP - c0)
        img_core[:nr, :ncol] = image[b, r0:r0 + nr, c0:c0 + ncol]

        core_labs = []
        for lab in labs:
            sl = np.zeros((NPATH, 128, PF), np.float32)
            vr = min(128, CH - r0)
            sl[:, :vr, :COLS_H] = lab[b, :, r0:r0 + vr, c0:c0 + COLS_H]
            sl = sl[perm].reshape(NG, GSZ, 128, PF)
            sl = np.ascontiguousarray(sl.transpose(0, 2, 1, 3)).reshape(NG, 128, GF)
            core_labs.append(sl)

        in_maps.append({"img": img_core,
                        "pos_lab": core_labs[0],
                        "neg_lab": core_labs[1]})
    return in_maps


_LAST_RESULTS = None


def kernel(pred_output, pos_label, neg_label, path_indices=None):
    global _LAST_RESULTS
    from concourse.bass_utils import run_bass_kernel_spmd

    nc = _get_program()
    in_maps = _host_prep(pred_output, pos_label, neg_label)

    res = run_bass_kernel_spmd(nc, in_maps, list(range(NCORES)))
    _LAST_RESULTS = res

    tp = tn = dp = dn = np.float64(0.0)
    for core in range(NCORES):
        o = np.asarray(res.results[core]["out"], dtype=np.float64)
        tp += o[:, 0:NG].sum()
        tn += o[:, NG:2 * NG].sum()
        dp += o[:, 2 * NG:3 * NG].sum()
        dn += o[:, 3 * NG:4 * NG].sum()

    pos_loss = -tp / (dp + EPS)
    neg_loss = -tn / (dn + EPS)
    return np.float32((pos_loss + neg_loss) / 2.0)


# revision 24
# speedup vs baseline: 1.2188x; 1.2188x over previous
"""Trainium2 Bass kernel for nn_CBDLoss (path-affinity cross-entropy loss).

Key insight: the int64 "gather" tables are just shifted 247x238 crop windows of
the 256x256 image. Each of the 152 path channels' affinity is
    aff[b, p, m] = 1 - max over path atoms (dy,dx) of image[b, r+dy, 9+dx+c]
with m = r*238 + c. So the whole gather+maxpool is a set of window-shifted
elementwise maxes, shareable across paths via a prefix trie (727 max ops
instead of 2134 raw path-steps).

Sharding (uniform SPMD, one program for all 8 cores): split the output grid by
(batch b, row-tile t, col-half h) -> 2*2*2 = 8 cores. Every core runs the same
global trie on its [128 rows x 120 cols] slice; padded rows/cols are masked by
zero labels (log terms stay finite, so 0*log = 0 contributes nothing).

Loss reduction is fused on-chip:
  logpos = Ln(-mx + (1+eps)),  logneg = Ln(mx + eps)        (ScalarE, LUT)
  numer += sum(label * log...)  via tensor_tensor_reduce     (VectorE, fused)
  denom += sum(label)           via activation Copy+accum    (ScalarE, fused)
Each core returns a [128, 32] block of per-partition partial sums; the host
does the tiny final reduction and the scalar loss formula.
"""

import os
import sys

import numpy as np

for _p in ("/opt/trn_rl_repo", "/root/.axon_site/_ro/trn_rl_repo"):
    if os.path.isdir(_p) and _p not in sys.path:
        sys.path.insert(0, _p)

RADIUS = 10
CROP = 256
EPS = 1e-5
B = 2
RF = RADIUS - 1          # 9
CH = CROP - RF           # 247 output rows
CW = CROP - 2 * RF       # 238 output cols
M = CH * CW              # 58786

NCORES = 8
NPATH = 152
NG = 8                   # path groups
GSZ = NPATH // NG        # 19 paths per group
PF = 120                 # per-path free width on a core (119 valid + 1 pad col)
GF = GSZ * PF            # 2280
COLS_H = 119             # valid cols per col-half
IMG_R = 138              # per-core image slice rows (128 + 9 halo + 1 pad)
IMG_C = 138              # per-core image slice cols (120 + 18 halo)
OUTW = 4 * NG            # out columns: [ttr_pos | ttr_neg | den_pos | den_neg]


def _gen_paths():
    """Replicates reference._get_all_dir_paths ordering -> 152 atom lists."""
    by_len = [[] for _ in range(RADIUS * 4)]
    search_dirs = [(0, x) for x in range(1, RADIUS)]
    for y in range(1, RADIUS):
        for x in range(-RADIUS + 1, RADIUS):
            if x * x + y * y < RADIUS ** 2:
                search_dirs.append((y, x))
    for d in search_dirs:
        length_sq = d[0] ** 2 + d[1] ** 2
        coords = []
        min_y, max_y = sorted((0, d[0]))
        min_x, max_x = sorted((0, d[1]))
        for y in range(min_y, max_y + 1):
            for x in range(min_x, max_x + 1):
                if (d[0] * x - d[1] * y) ** 2 / length_sq < 1:
                    coords.append((y, x))
        coords.sort(key=lambda c: -abs(c[0]) - abs(c[1]))
        by_len[len(coords)].append(coords)
    paths = []
    for g in by_len:
        paths.extend(g)
    assert len(paths) == NPATH
    return paths


def _trie():
    """Build the shared-prefix trie over canonically-ordered atom lists.

    Returns (order, term_of_node, perm):
      order: DFS list of op-nodes (tuples of atoms, depth >= 2)
      term_of_node: node -> terminal slot index (DFS terminal order) or None
      perm: perm[slot] = original path index whose labels go in that slot
    """
    paths = _gen_paths()
    cpaths = [tuple(sorted(p, key=lambda c: (abs(c[0]) + abs(c[1]), c[0], c[1])))
              for p in paths]
    nodes = set()
    for cp in cpaths:
        for i in range(2, len(cp) + 1):
            nodes.add(cp[:i])
    children = {}
    for n in nodes:
        children.setdefault(n[:-1], []).append(n)
    for k in children:
        children[k].sort()
    term_set = set(cpaths)
    assert len(term_set) == NPATH

    order = []
    term_order = []
    stack = list(reversed(children.get(((0, 0),), [])))
    # iterative DFS preserving recursive order
    def dfs(node):
        order.append(node)
        if node in term_set:
            term_order.append(node)
        for c in children.get(node, []):
            dfs(c)
    sys.setrecursionlimit(10000)
    for c in children.get(((0, 0),), []):
        dfs(c)
    assert len(order) == len(nodes) and len(term_order) == NPATH

    term_idx = {t: i for i, t in enumerate(term_order)}
    term_of_node = {n: term_idx.get(n) for n in order}
    perm = sorted(range(NPATH), key=lambda j: term_idx[cpaths[j]])
    return order, term_of_node, perm


_PERM = None
_PROG = None


def _build_program():
    import concourse.bass as bass
    import concourse.mybir as mybir
    import concourse.tile as tile

    dt = mybir.dt
    f32 = dt.float32
    order, term_of_node, _ = _trie()

    bf16 = dt.bfloat16
    nc = bass.Bass()
    img = nc.declare_dram_parameter("img", [IMG_R, IMG_C], bf16, isOutput=False)
    pos_lab = nc.declare_dram_parameter("pos_lab", [NG, 128, GF], f32, isOutput=False)
    neg_lab = nc.declare_dram_parameter("neg_lab", [NG, 128, GF], f32, isOutput=False)
    out = nc.declare_dram_parameter("out", [128, OUTW], f32, isOutput=True)

    with tile.TileContext(nc) as tc:
        from contextlib import ExitStack
        with ExitStack() as ctx:
            const_pool = ctx.enter_context(tc.tile_pool(name="const", bufs=1))
            trie_pool = ctx.enter_context(tc.tile_pool(name="trie", bufs=24))
            mxg_pool = ctx.enter_context(tc.tile_pool(name="mxg", bufs=2))
            lab_pool = ctx.enter_context(tc.tile_pool(name="lab", bufs=2))
            log_pool = ctx.enter_context(tc.tile_pool(name="log", bufs=2))
            dsc_pool = ctx.enter_context(tc.tile_pool(name="dsc", bufs=2))
            accs_pool = ctx.enter_context(tc.tile_pool(name="accs", bufs=8))

            # All 10 row-shifted image copies in ONE tile via ONE dma:
            # imgall_ev[p, dy*IMG_C + c] = img[p+dy, c]  (affine src pattern);
            # imgall_od is the same shifted one column so every atom slice
            # can start at an even element offset (DVE 2x bf16 mode needs
            # 4-byte-aligned step-1 operands).
            # Both column-shift copies (even at free offset 0, odd-shifted at
            # RADIUS*IMG_C) in ONE 3-dim DMA: the (dy, c) dims merge because
            # the host image stride equals IMG_C. The odd copy's col 137 of
            # each dy-block wraps to the next image row - never read (odd
            # atom slices only reach col 135).
            from concourse.bass_types import AP as BassAP
            imgall = const_pool.tile([128, 2 * RADIUS * IMG_C], bf16, tag="imgall")
            img_src = BassAP(img[:, :].tensor, 0,
                             [[IMG_C, 128], [1, 2], [1, RADIUS * IMG_C]])
            nc.sync.dma_start(
                imgall[:, :].rearrange("p (s k) -> p s k", s=2),
                img_src)
            bias_pos = const_pool.tile([128, 1], f32, tag="bias_pos")
            nc.vector.memset(bias_pos[:, :], 1.0 + EPS)
            bias_neg = const_pool.tile([128, 1], f32, tag="bias_neg")
            nc.vector.memset(bias_neg[:, :], EPS)

            # write-only dump tiles for the mandatory full-size outputs of
            # TTR / Copy+accum; allocated once so no pool-slot recycling
            # (slot reuse makes Tile emit extra per-instruction sem waits,
            # and walrus caps sync waits per instruction).
            dump_vp = const_pool.tile([128, GF], f32, tag="dump_vp")
            dump_vn = const_pool.tile([128, GF], f32, tag="dump_vn")

            acc_vp = const_pool.tile([128, NG], f32, tag="acc_vp")
            acc_vn = const_pool.tile([128, NG], f32, tag="acc_vn")
            acc_sp_tiles = []
            acc_sn_tiles = []

            def atom_ap(a):
                dy, dx = a
                s = dy * IMG_C + RF + dx
                if s % 2 == 0:
                    return imgall[:, s:s + PF]
                return imgall[:, RADIUS * IMG_C + s - 1:RADIUS * IMG_C + s - 1 + PF]

            Ln = mybir.ActivationFunctionType.Ln
            Copy = mybir.ActivationFunctionType.Copy
            mult = mybir.AluOpType.mult
            add = mybir.AluOpType.add

            node_val = {}
            mxg_tile = None

            def consume_group(g, mxg):
                labp = lab_pool.tile([128, GF], f32, tag="labp")
                nc.sync.dma_start(labp[:, :], pos_lab[g, :, :])
                labn = lab_pool.tile([128, GF], f32, tag="labn")
                nc.sync.dma_start(labn[:, :], neg_lab[g, :, :])

                logp = log_pool.tile([128, GF], f32, tag="logp")
                nc.scalar.activation(logp[:, :], mxg[:, :], Ln,
                                     bias=bias_neg[:, 0:1], scale=1.0)
                logn = log_pool.tile([128, GF], f32, tag="logn")
                nc.scalar.activation(logn[:, :], mxg[:, :], Ln,
                                     bias=bias_pos[:, 0:1], scale=-1.0)

                nc.vector.scalar_tensor_tensor(
                    out=dump_vp[:, :], in0=labp[:, :], scalar=1.0,
                    in1=logp[:, :], op0=mult, op1=mult,
                    accum_out=acc_vp[:, g:g + 1])
                nc.vector.scalar_tensor_tensor(
                    out=dump_vn[:, :], in0=labn[:, :], scalar=1.0,
                    in1=logn[:, :], op0=mult, op1=mult,
                    accum_out=acc_vn[:, g:g + 1])

                dsp = dsc_pool.tile([128, GF], f32, tag="dsp")
                asp = accs_pool.tile([128, 1], f32, tag="asp")
                nc.scalar.activation(dsp[:, :], labp[:, :], Copy,
                                     accum_out=asp[:, 0:1])
                acc_sp_tiles.append(asp)
                dsn = dsc_pool.tile([128, GF], f32, tag="dsn")
                asn = accs_pool.tile([128, 1], f32, tag="asn")
                nc.scalar.activation(dsn[:, :], labn[:, :], Copy,
                                     accum_out=asn[:, 0:1])
                acc_sn_tiles.append(asn)

            for node in order:
                parent = node[:-1]
                pin = atom_ap(parent[0]) if len(parent) == 1 else node_val[parent]
                ain = atom_ap(node[-1])
                slot = term_of_node[node]
                if slot is None:
                    dest_tile = trie_pool.tile([128, PF], bf16, tag="trie")
                    dest = dest_tile[:, :]
                else:
                    g, s = divmod(slot, GSZ)
                    if s == 0:
                        mxg_tile = mxg_pool.tile([128, GF], bf16, tag="mxg")
                    dest = mxg_tile[:, s * PF:(s + 1) * PF]
                nc.vector.tensor_tensor(dest, pin, ain, mybir.AluOpType.min)
                node_val[node] = dest
                if slot is not None and slot % GSZ == GSZ - 1:
                    consume_group(slot // GSZ, mxg_tile)

            nc.sync.dma_start(out[:, 0:NG], acc_vp[:, :])
            nc.sync.dma_start(out[:, NG:2 * NG], acc_vn[:, :])
            for g in range(NG):
                nc.sync.dma_start(out[:, 2 * NG + g:2 * NG + g + 1],
                                  acc_sp_tiles[g][:, 0:1])
                nc.sync.dma_start(out[:, 3 * NG + g:3 * NG + g + 1],
                                  acc_sn_tiles[g][:, 0:1])

    _hoist_excess_waits(nc, mybir)
    _replace_sem_range_clear(nc, mybir)
    return nc


def _replace_sem_range_clear(nc, mybir):
    """Tile's epilogue clears its semaphore range with a raw-ISA EVENT_SEM
    RANGE_CLEAR on Pool, which this walrus build rejects ("ISA wrong length").
    Replace it with per-semaphore EventSemaphore writes of 0."""
    for fn in nc.m.functions:
        for b in fn.blocks:
            out_list = []
            for inst in b.instructions:
                if type(inst).__name__ == "InstISA":
                    d = inst.ant_dict
                    if isinstance(d, dict) and "range_first" in d:
                        si = inst.sync_info
                        waits = list(si.on_wait) if si and si.on_wait else []
                        base_updates = list(si.on_update) if si and si.on_update else []
                        for k, sem in enumerate(
                                range(d["range_first"], d["range_last"] + 1)):
                            ev = mybir.InstEventSemaphore(
                                name=f"{inst.name}-semclr-{sem}")
                            ev.engine = inst.engine
                            upd = [mybir.SyncUpdate(
                                sync_type="semaphore", id=sem,
                                ant_name=f"semclr_{sem}",
                                update_mode="sem-wr-imm", update_value=0)]
                            if k == len(range(d["range_first"], d["range_last"] + 1)) - 1:
                                upd.extend(base_updates)
                            ev.sync_info = mybir.SyncInfo(
                                on_wait=waits if k == 0 else [],
                                on_update=upd)
                            out_list.append(ev)
                        continue
                out_list.append(inst)
            b.instructions[:] = out_list


def _hoist_excess_waits(nc, mybir):
    """Walrus caps semaphore waits per hardware instruction (1 for ACT, 2 for
    TT-family).  Move excess waits onto standalone NoOps on the same engine,
    inserted just before the over-limit instruction."""
    LIMITS = {}   # one wait per instruction across the board
    uid = 0
    for fn in nc.m.functions:
        for b in fn.blocks:
            out_list = []
            for inst in b.instructions:
                si = getattr(inst, "sync_info", None)
                if si is not None and si.on_wait:
                    lim = LIMITS.get(type(inst).__name__, 1)
                    waits = list(si.on_wait)
                    if len(waits) > lim:
                        keep = waits[-lim:] if lim else []
                        for w in waits[:len(waits) - lim]:
                            nop = mybir.InstDrain(name=f"I-waithoist-{uid}")
                            uid += 1
                            nop.engine = inst.engine
                            nop.sync_info = mybir.SyncInfo(
                                on_wait=[w], on_update=[])
                            out_list.append(nop)
                        inst.sync_info = mybir.SyncInfo(
                            on_wait=keep, on_update=list(si.on_update or []))
                out_list.append(inst)
            b.instructions[:] = out_list


def _get_program():
    global _PROG, _PERM
    if _PROG is None:
        _, _, _PERM = _trie()
        _PROG = _build_program()
    return _PROG


def _host_prep(pred_output, pos_label, neg_label):
    """Build per-core input maps."""
    global _PERM
    import ml_dtypes
    image = np.ascontiguousarray(
        1.0 - np.asarray(pred_output, dtype=np.float32).reshape(B, CROP, CROP)
    ).astype(ml_dtypes.bfloat16)
    labs = [np.asarray(pos_label, dtype=np.float32).reshape(B, NPATH, CH, CW),
            np.asarray(neg_label, dtype=np.float32).reshape(B, NPATH, CH, CW)]
    perm = _PERM

    in_maps = []
    for core in range(NCORES):
        b, t, h = core >> 2, (core >> 1) & 1, core & 1
        r0, c0 = t * 128, h * COLS_H

        img_core = np.zeros((IMG_R, IMG_C), image.dtype)
        nr = min(IMG_R - 1, CROP - r0)
        ncol = min(IMG_C, CROP - c0)
        img_core[:nr, :ncol] = image[b, r0:r0 + nr, c0:c0 + ncol]

        core_labs = []
        for lab in labs:
            sl = np.zeros((NPATH, 128, PF), np.float32)
            vr = min(128, CH - r0)
            sl[:, :vr, :COLS_H] = lab[b, :, r0:r0 + vr, c0:c0 + COLS_H]
            sl = sl[perm].reshape(NG, GSZ, 128, PF)
            sl = np.ascontiguousarray(sl.transpose(0, 2, 1, 3)).reshape(NG, 128, GF)
            core_labs.append(sl)

        in_maps.append({"img": img_core,
                        "pos_lab": core_labs[0],
                        "neg_lab": core_labs[1]})
    return in_maps


_LAST_RESULTS = None


def kernel(pred_output, pos_label, neg_label, path_indices=None):
    global _LAST_RESULTS
    from concourse.bass_utils import run_bass_kernel_spmd

    nc = _get_program()
    in_maps = _host_prep(pred_output, pos_label, neg_label)

    res = run_bass_kernel_spmd(nc, in_maps, list(range(NCORES)))
    _LAST_RESULTS = res

    tp = tn = dp = dn = np.float64(0.0)
    for core in range(NCORES):
        o = np.asarray(res.results[core]["out"], dtype=np.float64)
        tp += o[:, 0:NG].sum()
        tn += o[:, NG:2 * NG].sum()
        dp += o[:, 2 * NG:3 * NG].sum()
        dn += o[:, 3 * NG:4 * NG].sum()

    pos_loss = -tp / (dp + EPS)
    neg_loss = -tn / (dn + EPS)
    return np.float32((pos_loss + neg_loss) / 2.0)


# revision 28
# speedup vs baseline: 1.8954x; 1.5552x over previous
"""Trainium2 Bass kernel for nn_CBDLoss (path-affinity cross-entropy loss).

Key insight: the int64 "gather" tables are just shifted 247x238 crop windows of
the 256x256 image. Each of the 152 path channels' affinity is
    aff[b, p, m] = 1 - max over path atoms (dy,dx) of image[b, r+dy, 9+dx+c]
with m = r*238 + c. So the whole gather+maxpool is a set of window-shifted
elementwise maxes, shareable across paths via a prefix trie (727 max ops
instead of 2134 raw path-steps).

Sharding (uniform SPMD, one program for all 8 cores): split the output grid by
(batch b, row-tile t, col-half h) -> 2*2*2 = 8 cores. Every core runs the same
global trie on its [128 rows x 120 cols] slice; padded rows/cols are masked by
zero labels (log terms stay finite, so 0*log = 0 contributes nothing).

Loss reduction is fused on-chip:
  logpos = Ln(-mx + (1+eps)),  logneg = Ln(mx + eps)        (ScalarE, LUT)
  numer += sum(label * log...)  via tensor_tensor_reduce     (VectorE, fused)
  denom += sum(label)           via activation Copy+accum    (ScalarE, fused)
Each core returns a [128, 32] block of per-partition partial sums; the host
does the tiny final reduction and the scalar loss formula.
"""

import os
import sys

import numpy as np

for _p in ("/opt/trn_rl_repo", "/root/.axon_site/_ro/trn_rl_repo"):
    if os.path.isdir(_p) and _p not in sys.path:
        sys.path.insert(0, _p)

RADIUS = 10
CROP = 256
EPS = 1e-5
B = 2
RF = RADIUS - 1          # 9
CH = CROP - RF           # 247 output rows
CW = CROP - 2 * RF       # 238 output cols
M = CH * CW              # 58786

NCORES = 8
NPATH = 152
NG = 8                   # path groups
GSZ = NPATH // NG        # 19 paths per group
PF = 120                 # per-path free width on a core (119 valid + 1 pad col)
GF = GSZ * PF            # 2280
COLS_H = 119             # valid cols per col-half
IMG_R = 138              # per-core image slice rows (128 + 9 halo + 1 pad)
IMG_C = 138              # per-core image slice cols (120 + 18 halo)
OUTW = 4 * NG            # out columns: [ttr_pos | ttr_neg | den_pos | den_neg]


def _gen_paths():
    """Replicates reference._get_all_dir_paths ordering -> 152 atom lists."""
    by_len = [[] for _ in range(RADIUS * 4)]
    search_dirs = [(0, x) for x in range(1, RADIUS)]
    for y in range(1, RADIUS):
        for x in range(-RADIUS + 1, RADIUS):
            if x * x + y * y < RADIUS ** 2:
                search_dirs.append((y, x))
    for d in search_dirs:
        length_sq = d[0] ** 2 + d[1] ** 2
        coords = []
        min_y, max_y = sorted((0, d[0]))
        min_x, max_x = sorted((0, d[1]))
        for y in range(min_y, max_y + 1):
            for x in range(min_x, max_x + 1):
                if (d[0] * x - d[1] * y) ** 2 / length_sq < 1:
                    coords.append((y, x))
        coords.sort(key=lambda c: -abs(c[0]) - abs(c[1]))
        by_len[len(coords)].append(coords)
    paths = []
    for g in by_len:
        paths.extend(g)
    assert len(paths) == NPATH
    return paths


def _trie():
    """Build the shared-prefix trie over canonically-ordered atom lists.

    Returns (order, term_of_node, perm):
      order: DFS list of op-nodes (tuples of atoms, depth >= 2)
      term_of_node: node -> terminal slot index (DFS terminal order) or None
      perm: perm[slot] = original path index whose labels go in that slot
    """
    paths = _gen_paths()
    cpaths = [tuple(sorted(p, key=lambda c: (abs(c[0]) + abs(c[1]), c[0], c[1])))
              for p in paths]
    nodes = set()
    for cp in cpaths:
        for i in range(2, len(cp) + 1):
            nodes.add(cp[:i])
    children = {}
    for n in nodes:
        children.setdefault(n[:-1], []).append(n)
    for k in children:
        children[k].sort()
    term_set = set(cpaths)
    assert len(term_set) == NPATH

    order = []
    term_order = []
    stack = list(reversed(children.get(((0, 0),), [])))
    # iterative DFS preserving recursive order
    def dfs(node):
        order.append(node)
        if node in term_set:
            term_order.append(node)
        for c in children.get(node, []):
            dfs(c)
    sys.setrecursionlimit(10000)
    for c in children.get(((0, 0),), []):
        dfs(c)
    assert len(order) == len(nodes) and len(term_order) == NPATH

    term_idx = {t: i for i, t in enumerate(term_order)}
    term_of_node = {n: term_idx.get(n) for n in order}
    perm = sorted(range(NPATH), key=lambda j: term_idx[cpaths[j]])
    return order, term_of_node, perm


def _plan():
    """Greedy pairwise CSE over path atom-sets: repeatedly merge the most
    co-occurring symbol pair into one min op. ~474 ops vs 727 for the
    canonical prefix trie. Returns (ops, perm): ops = ordered
    (dest, src0, src1, terminal_slot|None); perm[slot] = original path idx."""
    import itertools
    from collections import Counter
    paths = _gen_paths()
    cur = [set(map(tuple, p)) for p in paths]
    ops = []
    done = [False] * NPATH
    term_order = []
    while True:
        cnt = Counter()
        for s in cur:
            if len(s) >= 2:
                for pair in itertools.combinations(sorted(s, key=repr), 2):
                    cnt[pair] += 1
        if not cnt:
            break
        best = max(cnt.items(), key=lambda kv: (kv[1], repr(kv[0])))[0]
        x, y = best
        z = (x, y)
        slot = None
        for i, s in enumerate(cur):
            if x in s and y in s:
                s.discard(x)
                s.discard(y)
                s.add(z)
                if len(s) == 1 and not done[i]:
                    done[i] = True
                    slot = len(term_order)
                    term_order.append(i)
        ops.append((z, x, y, slot))
    assert all(done) and len(term_order) == NPATH

    # Demand-driven topological reorder: emit each path's missing deps in
    # slot order. Spreads terminals across the program (group 0 completes
    # ~op 36 instead of ~op 326) and cuts max-live intermediates 205 -> 63.
    deps = {z: (x, y) for (z, x, y, _) in ops}
    final_of = {term_order[s]: z for (z, _, _, s) in ops if s is not None}
    slot_of_final = {final_of[term_order[s]]: s for s in range(NPATH)}

    def is_atom(sym):
        return isinstance(sym[0], int)

    emitted = set()
    order2 = []

    def emit(sym):
        if is_atom(sym) or sym in emitted:
            return
        x, y = deps[sym]
        emit(x)
        emit(y)
        emitted.add(sym)
        order2.append(sym)

    sys.setrecursionlimit(10000)
    for slot in range(NPATH):
        f = final_of[term_order[slot]]
        assert f not in emitted
        emit(f)
        assert order2[-1] is f or order2[-1] == f
    ops2 = [(z, deps[z][0], deps[z][1], slot_of_final.get(z)) for z in order2]
    return ops2, term_order


_PERM = None
_PROG = None


def _build_program():
    import concourse.bass as bass
    import concourse.mybir as mybir
    import concourse.tile as tile

    dt = mybir.dt
    f32 = dt.float32
    plan_ops, term_order = _plan()

    bf16 = dt.bfloat16
    nc = bass.Bass()
    img = nc.declare_dram_parameter("img", [IMG_R, IMG_C], bf16, isOutput=False)
    pos_lab = nc.declare_dram_parameter("pos_lab", [NG, 128, GF], f32, isOutput=False)
    neg_lab = nc.declare_dram_parameter("neg_lab", [NG, 128, GF], f32, isOutput=False)
    out = nc.declare_dram_parameter("out", [128, OUTW], f32, isOutput=True)

    with tile.TileContext(nc) as tc:
        from contextlib import ExitStack
        with ExitStack() as ctx:
            const_pool = ctx.enter_context(tc.tile_pool(name="const", bufs=1))
            trie_pool = ctx.enter_context(tc.tile_pool(name="trie", bufs=72))
            mxg_pool = ctx.enter_context(tc.tile_pool(name="mxg", bufs=2))
            lab_pool = ctx.enter_context(tc.tile_pool(name="lab", bufs=2))
            log_pool = ctx.enter_context(tc.tile_pool(name="log", bufs=2))
            dsc_pool = ctx.enter_context(tc.tile_pool(name="dsc", bufs=2))
            accs_pool = ctx.enter_context(tc.tile_pool(name="accs", bufs=8))

            # All 10 row-shifted image copies in ONE tile via ONE dma:
            # imgall_ev[p, dy*IMG_C + c] = img[p+dy, c]  (affine src pattern);
            # imgall_od is the same shifted one column so every atom slice
            # can start at an even element offset (DVE 2x bf16 mode needs
            # 4-byte-aligned step-1 operands).
            # Both column-shift copies (even at free offset 0, odd-shifted at
            # RADIUS*IMG_C) in ONE 3-dim DMA: the (dy, c) dims merge because
            # the host image stride equals IMG_C. The odd copy's col 137 of
            # each dy-block wraps to the next image row - never read (odd
            # atom slices only reach col 135).
            from concourse.bass_types import AP as BassAP
            imgall = const_pool.tile([128, 2 * RADIUS * IMG_C], bf16, tag="imgall")
            img_src = BassAP(img[:, :].tensor, 0,
                             [[IMG_C, 128], [1, 2], [1, RADIUS * IMG_C]])
            nc.sync.dma_start(
                imgall[:, :].rearrange("p (s k) -> p s k", s=2),
                img_src)
            bias_pos = const_pool.tile([128, 1], f32, tag="bias_pos")
            nc.vector.memset(bias_pos[:, :], 1.0 + EPS)
            bias_neg = const_pool.tile([128, 1], f32, tag="bias_neg")
            nc.vector.memset(bias_neg[:, :], EPS)

            # write-only dump tiles for the mandatory full-size outputs of
            # TTR / Copy+accum; allocated once so no pool-slot recycling
            # (slot reuse makes Tile emit extra per-instruction sem waits,
            # and walrus caps sync waits per instruction).
            dump_vp = const_pool.tile([128, GF], f32, tag="dump_vp")
            dump_vn = const_pool.tile([128, GF], f32, tag="dump_vn")

            acc_vp = const_pool.tile([128, NG], f32, tag="acc_vp")
            acc_vn = const_pool.tile([128, NG], f32, tag="acc_vn")
            acc_sp_tiles = []
            acc_sn_tiles = []

            def atom_ap(a):
                dy, dx = a
                s = dy * IMG_C + RF + dx
                if s % 2 == 0:
                    return imgall[:, s:s + PF]
                return imgall[:, RADIUS * IMG_C + s - 1:RADIUS * IMG_C + s - 1 + PF]

            Ln = mybir.ActivationFunctionType.Ln
            Copy = mybir.ActivationFunctionType.Copy
            mult = mybir.AluOpType.mult
            add = mybir.AluOpType.add

            node_val = {}
            mxg_tile = None

            def sym_ap(s):
                if isinstance(s[0], int):
                    return atom_ap(s)
                return node_val[s]

            def consume_group(g, mxg):
                labp = lab_pool.tile([128, GF], f32, tag="labp")
                nc.sync.dma_start(labp[:, :], pos_lab[g, :, :])
                labn = lab_pool.tile([128, GF], f32, tag="labn")
                nc.sync.dma_start(labn[:, :], neg_lab[g, :, :])

                logp = log_pool.tile([128, GF], f32, tag="logp")
                nc.scalar.activation(logp[:, :], mxg[:, :], Ln,
                                     bias=bias_neg[:, 0:1], scale=1.0)
                logn = log_pool.tile([128, GF], f32, tag="logn")
                nc.scalar.activation(logn[:, :], mxg[:, :], Ln,
                                     bias=bias_pos[:, 0:1], scale=-1.0)

                nc.vector.scalar_tensor_tensor(
                    out=dump_vp[:, :], in0=labp[:, :], scalar=1.0,
                    in1=logp[:, :], op0=mult, op1=mult,
                    accum_out=acc_vp[:, g:g + 1])
                nc.vector.scalar_tensor_tensor(
                    out=dump_vn[:, :], in0=labn[:, :], scalar=1.0,
                    in1=logn[:, :], op0=mult, op1=mult,
                    accum_out=acc_vn[:, g:g + 1])

                dsp = dsc_pool.tile([128, GF], f32, tag="dsp")
                asp = accs_pool.tile([128, 1], f32, tag="asp")
                nc.scalar.activation(dsp[:, :], labp[:, :], Copy,
                                     accum_out=asp[:, 0:1])
                acc_sp_tiles.append(asp)
                dsn = dsc_pool.tile([128, GF], f32, tag="dsn")
                asn = accs_pool.tile([128, 1], f32, tag="asn")
                nc.scalar.activation(dsn[:, :], labn[:, :], Copy,
                                     accum_out=asn[:, 0:1])
                acc_sn_tiles.append(asn)

            for (z, x, y, slot) in plan_ops:
                if slot is None:
                    dest_tile = trie_pool.tile([128, PF], bf16, tag="trie")
                    dest = dest_tile[:, :]
                else:
                    g, sl = divmod(slot, GSZ)
                    if sl == 0:
                        mxg_tile = mxg_pool.tile([128, GF], bf16, tag="mxg")
                    dest = mxg_tile[:, sl * PF:(sl + 1) * PF]
                nc.vector.tensor_tensor(dest, sym_ap(x), sym_ap(y),
                                        mybir.AluOpType.min)
                node_val[z] = dest
                if slot is not None and slot % GSZ == GSZ - 1:
                    consume_group(slot // GSZ, mxg_tile)

            nc.sync.dma_start(out[:, 0:NG], acc_vp[:, :])
            nc.sync.dma_start(out[:, NG:2 * NG], acc_vn[:, :])
            for g in range(NG):
                nc.sync.dma_start(out[:, 2 * NG + g:2 * NG + g + 1],
                                  acc_sp_tiles[g][:, 0:1])
                nc.sync.dma_start(out[:, 3 * NG + g:3 * NG + g + 1],
                                  acc_sn_tiles[g][:, 0:1])

    _hoist_excess_waits(nc, mybir)
    _replace_sem_range_clear(nc, mybir)
    return nc


def _replace_sem_range_clear(nc, mybir):
    """Tile's epilogue clears its semaphore range with a raw-ISA EVENT_SEM
    RANGE_CLEAR on Pool, which this walrus build rejects ("ISA wrong length").
    Replace it with per-semaphore EventSemaphore writes of 0."""
    for fn in nc.m.functions:
        for b in fn.blocks:
            out_list = []
            for inst in b.instructions:
                if type(inst).__name__ == "InstISA":
                    d = inst.ant_dict
                    if isinstance(d, dict) and "range_first" in d:
                        si = inst.sync_info
                        waits = list(si.on_wait) if si and si.on_wait else []
                        base_updates = list(si.on_update) if si and si.on_update else []
                        for k, sem in enumerate(
                                range(d["range_first"], d["range_last"] + 1)):
                            ev = mybir.InstEventSemaphore(
                                name=f"{inst.name}-semclr-{sem}")
                            ev.engine = inst.engine
                            upd = [mybir.SyncUpdate(
                                sync_type="semaphore", id=sem,
                                ant_name=f"semclr_{sem}",
                                update_mode="sem-wr-imm", update_value=0)]
                            if k == len(range(d["range_first"], d["range_last"] + 1)) - 1:
                                upd.extend(base_updates)
                            ev.sync_info = mybir.SyncInfo(
                                on_wait=waits if k == 0 else [],
                                on_update=upd)
                            out_list.append(ev)
                        continue
                out_list.append(inst)
            b.instructions[:] = out_list


def _hoist_excess_waits(nc, mybir):
    """Walrus caps semaphore waits per hardware instruction (1 for ACT, 2 for
    TT-family).  Move excess waits onto standalone NoOps on the same engine,
    inserted just before the over-limit instruction."""
    LIMITS = {}   # one wait per instruction across the board
    uid = 0
    for fn in nc.m.functions:
        for b in fn.blocks:
            out_list = []
            for inst in b.instructions:
                si = getattr(inst, "sync_info", None)
                if si is not None and si.on_wait:
                    lim = LIMITS.get(type(inst).__name__, 1)
                    waits = list(si.on_wait)
                    if len(waits) > lim:
                        keep = waits[-lim:] if lim else []
                        for w in waits[:len(waits) - lim]:
                            nop = mybir.InstDrain(name=f"I-waithoist-{uid}")
                            uid += 1
                            nop.engine = inst.engine
                            nop.sync_info = mybir.SyncInfo(
                                on_wait=[w], on_update=[])
                            out_list.append(nop)
                        inst.sync_info = mybir.SyncInfo(
                            on_wait=keep, on_update=list(si.on_update or []))
                out_list.append(inst)
            b.instructions[:] = out_list


def _get_program():
    global _PROG, _PERM
    if _PROG is None:
        _, _PERM = _plan()
        _PROG = _build_program()
    return _PROG


def _host_prep(pred_output, pos_label, neg_label):
    """Build per-core input maps."""
    global _PERM
    import ml_dtypes
    image = np.ascontiguousarray(
        1.0 - np.asarray(pred_output, dtype=np.float32).reshape(B, CROP, CROP)
    ).astype(ml_dtypes.bfloat16)
    labs = [np.asarray(pos_label, dtype=np.float32).reshape(B, NPATH, CH, CW),
            np.asarray(neg_label, dtype=np.float32).reshape(B, NPATH, CH, CW)]
    perm = _PERM

    in_maps = []
    for core in range(NCORES):
        b, t, h = core >> 2, (core >> 1) & 1, core & 1
        r0, c0 = t * 128, h * COLS_H

        img_core = np.zeros((IMG_R, IMG_C), image.dtype)
        nr = min(IMG_R - 1, CROP - r0)
        ncol = min(IMG_C, CROP - c0)
        img_core[:nr, :ncol] = image[b, r0:r0 + nr, c0:c0 + ncol]

        core_labs = []
        for lab in labs:
            sl = np.zeros((NPATH, 128, PF), np.float32)
            vr = min(128, CH - r0)
            sl[:, :vr, :COLS_H] = lab[b, :, r0:r0 + vr, c0:c0 + COLS_H]
            sl = sl[perm].reshape(NG, GSZ, 128, PF)
            sl = np.ascontiguousarray(sl.transpose(0, 2, 1, 3)).reshape(NG, 128, GF)
            core_labs.append(sl)

        in_maps.append({"img": img_core,
                        "pos_lab": core_labs[0],
                        "neg_lab": core_labs[1]})
    return in_maps


_LAST_RESULTS = None


def kernel(pred_output, pos_label, neg_label, path_indices=None):
    global _LAST_RESULTS
    from concourse.bass_utils import run_bass_kernel_spmd

    nc = _get_program()
    in_maps = _host_prep(pred_output, pos_label, neg_label)

    res = run_bass_kernel_spmd(nc, in_maps, list(range(NCORES)))
    _LAST_RESULTS = res

    tp = tn = dp = dn = np.float64(0.0)
    for core in range(NCORES):
        o = np.asarray(res.results[core]["out"], dtype=np.float64)
        tp += o[:, 0:NG].sum()
        tn += o[:, NG:2 * NG].sum()
        dp += o[:, 2 * NG:3 * NG].sum()
        dn += o[:, 3 * NG:4 * NG].sum()

    pos_loss = -tp / (dp + EPS)
    neg_loss = -tn / (dn + EPS)
    return np.float32((pos_loss + neg_loss) / 2.0)
